# revision 13
# baseline (speedup 1.0000x reference)
"""AdaLoRA routed-LoRA kernel for 8 Trainium2 NeuronCores.

Problem (nn_AdaLoRA): per token t with expert index i:
    ds[t, :]  = slots[t, :] @ down_table[i]            # [1024] @ [1024, 16]
    out[t, :] = (ds[t, :] @ up_table[i]) / sqrt(16)    # [16] @ [16, 1024]

Sharding: data-parallel over batch (B=8 -> one batch row per core; LoRA
tables replicated on every core, f16). Per core: 256 tokens = 2 tiles
of 128 tokens (tokens on SBUF partitions). The kernel is HBM-gather
bound (~16MB of f16 table rows per core at ~410GB/s aggregate over the
16 DMA queues), so the structure keeps the queues saturated end to end
and hides all compute under the gather:

- gather issue order D0* D1* U0* U1* (indirect DMAs; element_offset
  slices the r-major rows into rank chunks). The first down chunk is
  only 2 ranks so the DVE pipeline starts ~3us earlier (its work is
  serialized and nearly fills the gather window); the last up chunk is
  only 2 ranks so the post-gather matmul tail is short.
- down projection per rank: odd ranks fused multiply+accumulate on DVE
  (scalar_tensor_tensor), even ranks as DVE multiply + Scalar-engine
  activation-accumulate, all accumulating f32. 1/sqrt(16) is folded
  into slots on the host.
- up projection on TensorE: lhsT_h = diag(ds[:, h]) built with a single
  tensor_scalar (identity x per-partition f32 scalar); out[t,:]
  accumulates 32 matmuls [128x128]x[128x512] per tile over the gathered
  rank-chunks into one [128,1024] f32 PSUM tile. PSUM is copied out on
  the Scalar engine and DMA'd per 512-column half so the copy/DMA
  overlap the final matmuls.
"""

import numpy as np

B, K, DIM, RANK, NE = 8, 256, 1024, 16, 4096
P = 128
N_TILE = K // P  # 2 token tiles per core
DROW = RANK * DIM  # 16384 f16 elements per table row
SCALE = 1.0 / 4.0  # 1/sqrt(RANK), folded into slots host-side
N_CORES = 8

# down-gather rank chunking: small first chunk starts DVE early
D_CHUNKS = {0: [4, 4, 8], 1: [8, 8]}
# up-gather rank chunking: tiny last chunk shortens the matmul tail
U_CHUNKS = {0: [4, 4, 4, 4], 1: [4, 4, 4, 4]}

_CACHE = {}


def _build():
    from concourse import bacc, bass, mybir, tile

    f32 = mybir.dt.float32
    f16 = mybir.dt.float16
    i32 = mybir.dt.int32
    mult = mybir.AluOpType.mult
    Copy = mybir.ActivationFunctionType.Copy

    nc = bacc.Bacc("TRN2", target_bir_lowering=False, dynamic_dma_scratch_size=65536)
    slots = nc.declare_dram_parameter("slots", [K, DIM], f16, isOutput=False)
    idx = nc.declare_dram_parameter("idx", [P, N_TILE], i32, isOutput=False)
    down16 = nc.declare_dram_parameter("down16", [NE, DROW], f16, isOutput=False)
    up16 = nc.declare_dram_parameter("up16", [NE, DROW], f16, isOutput=False)
    ident_c = nc.declare_dram_parameter("ident_c", [P, P], f16, isOutput=False)
    out = nc.declare_dram_parameter("out", [K, DIM], f16, isOutput=True)

    with tile.TileContext(nc) as tc:
        with (
            tc.tile_pool(name="io", bufs=2) as io_pool,
            tc.tile_pool(name="dg", bufs=1) as dg_pool,
            tc.tile_pool(name="prod", bufs=2) as pr_pool,
            tc.tile_pool(name="upg", bufs=7) as up_pool,
            tc.tile_pool(name="upg2", bufs=2) as up2_pool,
            tc.tile_pool(name="lhs", bufs=2) as lh_pool,
            tc.tile_pool(name="misc", bufs=1) as m_pool,
            tc.tile_pool(name="ob", bufs=1) as ob_pool,
            tc.tile_pool(name="ps", bufs=2, space="PSUM") as ps_pool,
        ):
            # ---- tiny index DMA first: it gates descriptor generation ----
            idx2 = m_pool.tile([P, N_TILE], i32)
            nc.sync.dma_start(out=idx2[:], in_=idx[:, :])
            slots_tiles = []
            for t in range(N_TILE):
                tok = slice(t * P, (t + 1) * P)
                slots16 = io_pool.tile([P, DIM], f16, tag="slots16")
                nc.sync.dma_start(out=slots16[:], in_=slots[tok, :])
                slots_tiles.append(slots16)
            ident = m_pool.tile([P, P], f16)
            nc.sync.dma_start(out=ident[:], in_=ident_c[:])

            # ---- gathers: down tile0, down tile1, up tile0, up tile1 ----
            dg_tiles = {}  # (t, chunk_index) -> (rank_start, nranks, tile)
            for t in range(N_TILE):
                r0 = 0
                for ci, nr in enumerate(D_CHUNKS[t]):
                    dg = dg_pool.tile([P, nr * DIM], f16, tag=f"dg_{t}_{ci}")
                    nc.gpsimd.indirect_dma_start(
                        out=dg[:],
                        out_offset=None,
                        in_=down16[:],
                        in_offset=bass.IndirectOffsetOnAxis(
                            ap=idx2[:, t : t + 1], axis=0
                        ),
                        element_offset=r0 * DIM,
                    )
                    dg_tiles[t, ci] = (r0, nr, dg)
                    r0 += nr
            upc_tiles = {}
            for t in range(N_TILE):
                r0 = 0
                for ci, nr in enumerate(U_CHUNKS[t]):
                    if nr == 4:
                        upc = up_pool.tile([P, 4 * DIM], f16, tag="upc")
                    else:
                        upc = up2_pool.tile([P, 2 * DIM], f16, tag="upc2")
                    nc.gpsimd.indirect_dma_start(
                        out=upc[:, : nr * DIM],
                        out_offset=None,
                        in_=up16[:],
                        in_offset=bass.IndirectOffsetOnAxis(
                            ap=idx2[:, t : t + 1], axis=0
                        ),
                        element_offset=r0 * DIM,
                    )
                    upc_tiles[t, ci] = (r0, nr, upc)
                    r0 += nr

            # ---- down projection + diagonal lhsT build ----
            scratch = m_pool.tile([P, DIM], f16)
            scratch2 = m_pool.tile([P, DIM], f16)
            lhsT_tiles = []
            for t in range(N_TILE):
                slots16 = slots_tiles[t]
                ds32 = io_pool.tile([P, RANK], f32, tag="ds32")
                for ci in range(len(D_CHUNKS[t])):
                    r0, nr, dg = dg_tiles[t, ci]
                    dch = dg[:].rearrange("p (r d) -> p r d", d=DIM)
                    # even ranks: DVE multiply feeding Scalar accumulate
                    prods = []
                    for rl in range(nr):
                        r = r0 + rl
                        if r % 2 == 0:
                            prod = pr_pool.tile([P, DIM], f16, tag=f"prod{rl % 4}")
                            nc.vector.tensor_tensor(
                                out=prod[:], in0=slots16[:], in1=dch[:, rl, :],
                                op=mult,
                            )
                            prods.append((r, prod))
                    # odd ranks: fused multiply+accumulate on DVE
                    for rl in range(nr):
                        r = r0 + rl
                        if r % 2 == 1:
                            nc.vector.scalar_tensor_tensor(
                                out=scratch[:],
                                in0=slots16[:],
                                scalar=1.0,
                                in1=dch[:, rl, :],
                                op0=mult,
                                op1=mult,
                                accum_out=ds32[:, r : r + 1],
                            )
                    for r, prod in prods:
                        nc.scalar.activation(
                            out=scratch2[:],
                            in_=prod[:],
                            func=Copy,
                            accum_out=ds32[:, r : r + 1],
                        )
                # lhsT_h = diag(ds32[:, h]) via identity x per-partition scalar
                lhsT = lh_pool.tile([P, RANK, P], f16, tag="lhsT")
                for h in range(RANK):
                    nc.vector.tensor_scalar(
                        out=lhsT[:, h, :],
                        in0=ident[:],
                        scalar1=ds32[:, h : h + 1],
                        scalar2=None,
                        op0=mult,
                    )
                lhsT_tiles.append(lhsT)

            # ---- up projection on TensorE + output ----
            for t in range(N_TILE):
                tok = slice(t * P, (t + 1) * P)
                out_psum = ps_pool.tile([P, DIM], f32, space="PSUM", tag="outp")
                nch = len(U_CHUNKS[t])
                for ci in range(nch):
                    r0, nr, upc = upc_tiles[t, ci]
                    for h in range(nr):
                        for n in range(2):
                            n0 = n * 512
                            nc.tensor.matmul(
                                out=out_psum[:, n0 : n0 + 512],
                                lhsT=lhsT_tiles[t][:, r0 + h, :],
                                rhs=upc[:, h * DIM + n0 : h * DIM + n0 + 512],
                                start=(ci == 0 and h == 0),
                                stop=(ci == nch - 1 and h == nr - 1),
                            )
                out_sb = ob_pool.tile([P, DIM], f16, tag="osb")
                for n in range(2):
                    n0 = n * 512
                    nc.scalar.copy(
                        out_sb[:, n0 : n0 + 512], out_psum[:, n0 : n0 + 512]
                    )
                    nc.sync.dma_start(
                        out=out[tok, n0 : n0 + 512], in_=out_sb[:, n0 : n0 + 512]
                    )
    nc.compile()
    return nc


def _get_nc():
    if "nc" not in _CACHE:
        _CACHE["nc"] = _build()
    return _CACHE["nc"]


def _prep_in_maps(slots, indices, down_proj_values, up_proj_values):
    slots = np.ascontiguousarray(
        (np.asarray(slots, dtype=np.float32) * SCALE).astype(np.float16)
    )
    indices = np.ascontiguousarray(np.asarray(indices).astype(np.int32))
    down16 = np.ascontiguousarray(
        np.asarray(down_proj_values, dtype=np.float32)
        .transpose(0, 2, 1)
        .reshape(NE, DROW)
        .astype(np.float16)
    )
    up16 = np.ascontiguousarray(
        np.asarray(up_proj_values, dtype=np.float32).reshape(NE, DROW).astype(np.float16)
    )
    ident_c = np.eye(P, dtype=np.float16)
    assert slots.shape == (B, K, DIM) and indices.shape == (B, K)
    in_maps = []
    for i in range(N_CORES):
        in_maps.append(
            {
                "slots": slots[i],
                "idx": indices[i].reshape(N_TILE, P).T.copy(),  # [P, N_TILE]
                "down16": down16,
                "up16": up16,
                "ident_c": ident_c,
            }
        )
    return in_maps


def _run(in_maps, trace=False):
    from concourse.bass_utils import run_bass_kernel_spmd

    nc = _get_nc()
    return run_bass_kernel_spmd(
        nc, in_maps, core_ids=list(range(N_CORES)), trace=trace
    )


def kernel(slots, indices, down_proj_values, up_proj_values):
    in_maps = _prep_in_maps(slots, indices, down_proj_values, up_proj_values)
    res = _run(in_maps)
    out = np.stack([res.results[i]["out"] for i in range(N_CORES)], axis=0)
    return out.astype(np.float32)
